# revision 28
# baseline (speedup 1.0000x reference)
"""Multi-head attention (B=2, S=2048, D=1024, H=16, dk=dv=64) on 8 TRN2 cores.

Sharding: core c -> batch b = c % 2, head-group g = c // 2 (heads 4g..4g+3).
Each core computes its 4 heads' attention for one batch plus the partial
output projection; the host sums the 4 partials per batch and adds bo plus
the (constant) V-bias term SCALE*bv@Wo -- softmax weights sum to 1, so the
V-bias contributes a constant vector that never needs to touch the device.

Device pipeline (weights/activations fp16, exp/ctx path bf16, all PSUM
accumulation fp32 -- measured end-to-end rel err ~2e-3 vs the 2e-2 gate).
Structured to keep the PE datapath gapless (HAM stays at 2.4GHz) and to
split the softmax exp between the scalar engine (exact, table-based) and
the vector engine (Schraudolph int16 fast-exp, fused single op):

  1. K proj: kt chunks [128,8,512] -> kwt[pair][dk,S] (heads pair-stacked on
     partitions, bias fused into the ACT PSUM->SBUF eviction). Q proj block 0.
  2. V proj in NATURAL orientation: lhsT = vt chunk (D on partitions),
     rhs = wv -> vw[t, 4*(dv+1)] directly, no PE transposes. The 65th column
     per head is a preloaded ones column (softmax denominator trick).
     Emitted as PE fillers inside the first attention block.
  3. Attention per (pair, block): 16 t-tile steps. Each step:
     scores[t,s] via 2 row-tiled concurrent 64-contraction matmuls into a
     double-buffered 2-bank PSUM tile; ONE exp ACT [128,1024] evicts both
     heads; ctx accumulation trails by 2 steps (2 matmuls into per-pair ct
     banks, ones column gives the denominator row). Fillers (V proj, next
     block's Q proj, previous block's out proj, normalize) drain one per step.
  4. Normalize: denominator row -> K=1 matmul broadcast -> DVE reciprocal ->
     DVE multiply into persistent ctxT. Out proj [s,D] partial with DVE
     eviction, DMA'd out.
"""
import os
import sys

sys.path.insert(0, "/opt/trn_rl_repo")
os.environ.setdefault("JAX_PLATFORMS", "axon,cpu")

from contextlib import ExitStack

import numpy as np

import concourse.bacc as bacc
import concourse.tile as tile
from concourse import mybir
from concourse.bass_utils import run_bass_kernel_spmd

FP32 = mybir.dt.float32
FP16 = mybir.dt.float16
BF16 = mybir.dt.bfloat16
I16 = mybir.dt.int16

B, S, D = 2, 2048, 1024
H, DK, DV = 16, 64, 64
N_CORES = 8
HPC = H // (N_CORES // B)  # heads per core = 4
P = 128
SBLK = 512                # s-block (free dim of scores matmuls)
NBLK = S // SBLK          # 4
NTT = S // P              # 16 t-tiles
NDC = D // P              # 8 contraction chunks
NV = HPC * (DV + 1)       # 260
SCALE = 1.0 / (DK * 2.0)  # folded into the softmax denominator on device
LAG = 3                   # ctx trails exp by LAG t-tile steps

# Schraudolph fast-exp, bf16 flavor: i16 = round(x * EXP_A16 + EXP_B16) is
# the bit pattern of bf16 ~= exp(x) * (1 + eps), |eps| < ~3% sawtooth. The
# global factor cancels in softmax; measured end-to-end rel err ~2.1e-3.
EXP_A16 = float((1 << 23) / np.log(2.0) / 65536.0)
EXP_B16 = float((127 * (1 << 23) - 486408) / 65536.0)
# steps whose exp runs on the DVE instead of ACT (k % 3 == 1 -> ~1/3)
def _exp_on_dve(k):
    return k % 3 == 1


def _build_nc():
    nc = bacc.Bacc("TRN2", target_bir_lowering=False, debug=False,
                   num_devices=N_CORES)
    d = {}
    for name, shape in [
        ("qt", [D, S]), ("kt", [D, S]), ("vt", [D, S]),
        ("wq", [D, 2 * P]), ("wk", [D, 2 * P]), ("wv", [D, 2 * P]),
        ("wo", [HPC * DV, D]),
    ]:
        d[name] = nc.dram_tensor(name, shape, FP16, kind="ExternalInput").ap()
    d["bqk"] = nc.dram_tensor("bqk", [P, 4], FP32, kind="ExternalInput").ap()
    out_d = nc.dram_tensor("out", [S, D], FP16, kind="ExternalOutput").ap()
    xt_view = {
        n: d[n].rearrange("(dc p) s -> p dc s", p=P)
        for n in ("qt", "kt", "vt")
    }

    with tile.TileContext(nc) as tc, ExitStack() as ctx:
        const = ctx.enter_context(tc.tile_pool(name="const", bufs=1))
        wpool = ctx.enter_context(tc.tile_pool(name="wpool", bufs=1))
        xtp = ctx.enter_context(tc.tile_pool(name="xtp", bufs=4))
        projp = ctx.enter_context(tc.tile_pool(name="projp", bufs=1))
        expp = ctx.enter_context(tc.tile_pool(name="expp", bufs=1))
        ctxp = ctx.enter_context(tc.tile_pool(name="ctxp", bufs=1))
        outp = ctx.enter_context(tc.tile_pool(name="outp", bufs=2))
        smallp = ctx.enter_context(tc.tile_pool(name="smallp", bufs=2))
        psum = ctx.enter_context(tc.tile_pool(name="psum", bufs=1, space="PSUM"))

        # ---- constants / weights (wk first: K projection starts the kernel) ----
        def load_w(sb, name, nj, pat="(dc p) m -> p dc m"):
            v = d[name].rearrange(pat, p=P)
            for j in range(nj):
                nc.sync.dma_start(sb[:, j, :], v[:, j, :])

        wk_sb = wpool.tile([P, NDC, 2 * P], FP16)
        load_w(wk_sb, "wk", NDC)
        bqk = const.tile([P, 4], FP32)
        nc.sync.dma_start(bqk[:], d["bqk"])
        wq_sb = wpool.tile([P, NDC, 2 * P], FP16)
        wv_sb = wpool.tile([P, NDC, 2 * P], FP16)
        wo_sb = wpool.tile([P, 2, D], FP16)

        # ---- persistent activation tiles ----
        qwt = [projp.tile([P, S], FP16, tag=f"qwt{p_}", name=f"qwt{p_}") for p_ in range(2)]
        kwt = [projp.tile([P, S], FP16, tag=f"kwt{p_}", name=f"kwt{p_}") for p_ in range(2)]
        vw = projp.tile([P, NTT, NV], BF16, tag="vw")
        # softmax-denominator ones column (once, strided over the 65-wide head slots)
        for hh in range(HPC):
            nc.vector.memset(vw[:, :, hh * (DV + 1) + DV], 1.0)
        ctx_t = [ctxp.tile([P, S], FP16, tag=f"ctx{p_}", name=f"ctx{p_}") for p_ in range(2)]

        def load_chunk(name, ci):
            # one dma_start per dc row: consumers unblock per-dc, and the 8
            # transfers spread across DMA queues
            xt = xtp.tile([P, NDC, SBLK], FP16, tag="xt", name="xt")
            for dc in range(NDC):
                nc.sync.dma_start(xt[:, dc, :],
                                  xt_view[name][:, dc, ci * SBLK:(ci + 1) * SBLK])
            return xt

        def proj_qk_pair(xt, w_sb, dst, bias_col, ci, pair, tag):
            """Project one head-pair of a 512-chunk into dst[pair][:, ci*SBLK:...]."""
            pq = psum.tile([P, SBLK], FP32, tag=tag, name="pq")
            for dc in range(NDC):
                nc.tensor.matmul(pq[:], lhsT=w_sb[:, dc, pair * P:(pair + 1) * P],
                                 rhs=xt[:, dc, :], start=(dc == 0), stop=(dc == NDC - 1))
            nc.scalar.activation(dst[pair][:, ci * SBLK:(ci + 1) * SBLK], pq[:],
                                 mybir.ActivationFunctionType.Identity,
                                 bias=bqk[:, bias_col + pair:bias_col + pair + 1])

        def proj_qk_piece(xt, w_sb, dst, bias_col, ci, pair, dc_range, pq_holder):
            if dc_range[0] == 0:
                pq_holder[pair] = psum.tile([P, SBLK], FP32, tag="pj", name="pq")
            pq = pq_holder[pair]
            for dc in dc_range:
                nc.tensor.matmul(pq[:], lhsT=w_sb[:, dc, pair * P:(pair + 1) * P],
                                 rhs=xt[:, dc, :], start=(dc == 0), stop=(dc == NDC - 1))
            if dc_range[-1] == NDC - 1:
                nc.scalar.activation(dst[pair][:, ci * SBLK:(ci + 1) * SBLK], pq[:],
                                     mybir.ActivationFunctionType.Identity,
                                     bias=bqk[:, bias_col + pair:bias_col + pair + 1])

        # V proj, natural orientation: one t-tile per call (8 matmuls, free=256).
        # vp holds 2 t-tiles per PSUM bank; evicted per t-tile by the DVE.
        vp_holder = [None]

        def proj_v_tt(vt_chunk, tt):
            par = tt % 2
            if par == 0:
                vp_holder[0] = psum.tile([P, 2, 2 * P], FP32, tag="po", name="vp")
            vp = vp_holder[0]
            off = (tt * P) % SBLK
            for dc in range(NDC):
                nc.tensor.matmul(vp[:, par, :],
                                 lhsT=vt_chunk[:, dc, off:off + P],
                                 rhs=wv_sb[:, dc, :],
                                 start=(dc == 0), stop=(dc == NDC - 1))
            # evict into the 65-wide head slots (dv 0:64 of each slot)
            nc.vector.tensor_copy(
                vw[:, tt, :].rearrange("p (h v) -> p h v", v=DV + 1)[:, :, 0:DV],
                vp[:, par, :].rearrange("p (h v) -> p h v", h=HPC))

        def attn_normalize_hp(pair, b, ct, hp):
            # ctx = ct[0:64] * (SCALE / ct[64]) row-broadcast, one head.
            # SCALE folds into the denominator (den * 1/SCALE) so it never
            # degrades the fp16 weight/activation paths.
            den = smallp.tile([1, SBLK], FP32, tag="den")
            nc.vector.tensor_scalar_mul(den[:], ct[hp][DV:DV + 1, :], 1.0 / SCALE)
            dbc = smallp.tile([DV, SBLK], FP32, tag="dbc")
            nc.gpsimd.partition_broadcast(dbc[:], den[:], channels=DV)
            rcp = smallp.tile([DV, SBLK], FP32, tag="rcp")
            nc.vector.reciprocal_approx_fast(rcp[:], dbc[:])
            nc.vector.tensor_mul(
                ctx_t[pair][hp * DV:(hp + 1) * DV, b * SBLK:(b + 1) * SBLK],
                ct[hp][0:DV, :], rcp[:])

        def norm_fillers(pair, b, ct):
            return [lambda h=hp: attn_normalize_hp(pair, b, ct, h)
                    for hp in range(2)]

        def out_proj_nh(b, st, nh, tag="po"):
            off = b * SBLK + st * P
            po = psum.tile([P, SBLK], FP32, tag=tag, name="po")
            for jc in range(2):
                nc.tensor.matmul(po[:],
                                 lhsT=ctx_t[jc][:, off:off + P],
                                 rhs=wo_sb[:, jc, nh * SBLK:(nh + 1) * SBLK],
                                 start=(jc == 0), stop=(jc == 1))
            ob = outp.tile([P, SBLK], FP16, tag="ob")
            nc.vector.tensor_copy(ob[:], po[:])
            nc.sync.dma_start(out_d[off:off + P, nh * SBLK:(nh + 1) * SBLK], ob[:])

        # ---- prologue: K full, Q block 0 ----
        for ci in range(NBLK):
            kt = load_chunk("kt", ci)
            if ci == 0:
                load_w(wq_sb, "wq", NDC)
            if ci == 1:
                load_w(wv_sb, "wv", NDC)
            if ci == 2:
                load_w(wo_sb, "wo", 2, "(jc p) n -> p jc n")
            proj_qk_pair(kt, wk_sb, kwt, 2, ci, 0, "pj")
            proj_qk_pair(kt, wk_sb, kwt, 2, ci, 1, "po")
        qt = load_chunk("qt", 0)
        proj_qk_pair(qt, wq_sb, qwt, 0, 0, 0, "pj")
        proj_qk_pair(qt, wq_sb, qwt, 0, 0, 1, "po")
        # vt chunks stream in during the first attention block
        vts = [load_chunk("vt", ci) for ci in range(2)]

        def interleave(a, bl):
            out = []
            for i in range(max(len(a), len(bl))):
                if i < len(a):
                    out.append(a[i])
                if i < len(bl):
                    out.append(bl[i])
            return out

        # ---- attention: (pair, block) segments of 16 t-tile steps ----
        def attn_segment(pair, b, ct, fillers):
            """scores(tt) -> exp(tt) -> ctx(tt-LAG), one filler per step."""
            exs = {}
            for k in range(NTT + LAG):
                if k < NTT:
                    tt = k
                    sc = psum.tile([P, 2, SBLK], FP32, tag="sc", name="sc", bufs=2)
                    for hp in range(2):
                        lo, hi = hp * DK, (hp + 1) * DK
                        nc.tensor.matmul(
                            sc[:, hp, :],
                            lhsT=kwt[pair][lo:hi, tt * P:(tt + 1) * P],
                            rhs=qwt[pair][lo:hi, b * SBLK:(b + 1) * SBLK],
                            start=True, stop=True)
                    ex = expp.tile([P, 2, SBLK], BF16, tag="exp", name="ex", bufs=LAG + 3)
                    if _exp_on_dve(k):
                        # fused fast-exp: int16(x*A + B) is bf16 ~= exp(x)
                        nc.vector.tensor_scalar(
                            ex[:].bitcast(I16), sc[:], EXP_A16, EXP_B16,
                            mybir.AluOpType.mult, mybir.AluOpType.add)
                    else:
                        nc.scalar.activation(ex[:], sc[:],
                                             mybir.ActivationFunctionType.Exp)
                    exs[k] = ex
                if fillers:
                    fillers.pop(0)()
                kc = k - LAG
                if kc >= 0:
                    ex = exs.pop(kc)
                    for hp in range(2):
                        hh = 2 * pair + hp
                        nc.tensor.matmul(
                            ct[hp][:], lhsT=vw[:, kc, hh * (DV + 1):(hh + 1) * (DV + 1)],
                            rhs=ex[:, hp, :],
                            start=(kc == 0), stop=(kc == NTT - 1))

        def attn_alloc():
            return [psum.tile([DV + 1, SBLK], FP32, tag=f"ct{hp}", name=f"ct{hp}")
                    for hp in range(2)]

        def v_filler(tt):
            def go():
                ci = tt // (SBLK // P)
                if tt % (SBLK // P) == 0 and ci + 2 < NBLK:
                    vts.append(load_chunk("vt", ci + 2))
                proj_v_tt(vts[ci], tt)
            return go

        prev_norm = []  # pair-1 normalize deferred into the next block
        for b in range(NBLK):
            have_next = b + 1 < NBLK
            # fillers for pair 0 segment: deferred normalize first (ct banks
            # are reused by this segment's ctx at step LAG)
            fill0 = list(prev_norm)
            prev_norm = []
            if b == 0:
                # V projection: all 16 t-tiles as fillers here — filler tt
                # runs at step tt, ctx(tt) consumes vw[tt] at step tt+LAG
                fill0 += [v_filler(tt) for tt in range(NTT)]
            else:
                for st in range(2):
                    for nh in range(2):
                        fill0.append(lambda s=st, n=nh, bb=b: out_proj_nh(bb - 1, s, n))
            ct = attn_alloc()
            attn_segment(0, b, ct, fill0)

            # fillers for pair 1 segment: normalize(0,b) first, then V/q-proj/
            # out-proj pieces
            fill1 = norm_fillers(0, b, ct)
            pp = []
            if have_next:
                qt = load_chunk("qt", b + 1)
                holder = [None, None]
                for pair_ in range(2):
                    for dcs in ([0, 1], [2, 3], [4, 5], [6, 7]):
                        pp.append(lambda xt=qt, p=pair_, r=tuple(dcs), h=holder:
                                  proj_qk_piece(xt, wq_sb, qwt, 0, b + 1, p, r, h))
            op = []
            if b > 0:
                for st in range(2, 4):
                    for nh in range(2):
                        op.append(lambda s=st, n=nh, bb=b: out_proj_nh(bb - 1, s, n))
            ct1 = attn_alloc()
            attn_segment(1, b, ct1, fill1 + interleave(pp, op))
            prev_norm = norm_fillers(1, b, ct1)
        for f in prev_norm:
            f()
        for st in range(4):
            for nh in range(2):
                out_proj_nh(NBLK - 1, st, nh, tag="po" if (st * 2 + nh) % 2 == 0 else "pj")

    nc.compile()
    return nc


_NC_CACHE = None


def _get_nc():
    global _NC_CACHE
    if _NC_CACHE is None:
        _NC_CACHE = _build_nc()
    return _NC_CACHE


def kernel(Q, K, V, Wq, bq, Wk, bk, Wv, bv, Wo, bo, _trace=False, _trace_kwargs=None):
    nc = _get_nc()
    f16 = np.float16
    qt_h = [np.ascontiguousarray(np.asarray(Q[b]).T.astype(f16)) for b in range(B)]
    kt_h = [np.ascontiguousarray(np.asarray(K[b]).T.astype(f16)) for b in range(B)]
    vt_h = [np.ascontiguousarray(np.asarray(V[b]).T.astype(f16)) for b in range(B)]

    in_maps = []
    for c in range(N_CORES):
        b, g = c % B, c // B
        hs = list(range(g * HPC, (g + 1) * HPC))
        wq_p = np.concatenate([Wq[h] for h in hs], axis=1)
        wk_p = np.concatenate([Wk[h] for h in hs], axis=1)
        wv_p = np.concatenate([Wv[h] for h in hs], axis=1)
        bqk_p = np.stack([
            np.concatenate([bq[hs[0]], bq[hs[1]]]),
            np.concatenate([bq[hs[2]], bq[hs[3]]]),
            np.concatenate([bk[hs[0]], bk[hs[1]]]),
            np.concatenate([bk[hs[2]], bk[hs[3]]]),
        ], axis=1)
        in_maps.append({
            "qt": qt_h[b], "kt": kt_h[b], "vt": vt_h[b],
            "wq": np.ascontiguousarray(wq_p.astype(f16)),
            "wk": np.ascontiguousarray(wk_p.astype(f16)),
            "wv": np.ascontiguousarray(wv_p.astype(f16)),
            "bqk": np.ascontiguousarray(bqk_p.astype(np.float32)),
            "wo": np.ascontiguousarray(
                Wo[g * HPC * DV:(g + 1) * HPC * DV].astype(f16)),
        })

    kw = {}
    if _trace:
        kw = dict(trace=True, **(_trace_kwargs or {}))
    res = run_bass_kernel_spmd(nc, in_maps, core_ids=list(range(N_CORES)), **kw)

    out = np.zeros((B, S, D), dtype=np.float32)
    for c in range(N_CORES):
        out[c % B] += res.results[c]["out"].astype(np.float32)
    # host-side constant terms: output bias + V-bias (softmax weights sum to 1,
    # so the V bias contributes SCALE * bv @ Wo, constant over (b, s))
    out += bo[None, None, :] + (SCALE * bv.reshape(-1)) @ Wo
    if _trace:
        return out, res
    return out


# revision 41
# speedup vs baseline: 1.2899x; 1.2899x over previous
"""Multi-head attention (B=2, S=2048, D=1024, H=16, dk=dv=64) on 8 TRN2 cores.

Sharding: core c -> batch b = c % 2, head-group g = c // 2 (heads 4g..4g+3).
Each core computes its 4 heads' attention for one batch plus the partial
output projection; the host sums the 4 partials per batch and adds bo plus
the (constant) V-bias term SCALE*bv@Wo -- softmax weights sum to 1, so the
V-bias contributes a constant vector that never needs to touch the device.

Device pipeline (weights/activations fp16, exp/ctx path bf16, all PSUM
accumulation fp32 -- measured end-to-end rel err ~2e-3 vs the 2e-2 gate).
Structured to keep the PE datapath gapless (HAM stays at 2.4GHz) and to
split the softmax exp between the scalar engine (exact, table-based) and
the vector engine (Schraudolph int16 fast-exp, fused single op):

  1. K proj: kt chunks [128,8,512] -> kwt[pair][dk,S] (heads pair-stacked on
     partitions, bias fused into the ACT PSUM->SBUF eviction). Q proj block 0.
  2. V proj in NATURAL orientation: lhsT = vt chunk (D on partitions),
     rhs = wv -> vw[t, 4*(dv+1)] directly, no PE transposes. The 65th column
     per head is a preloaded ones column (softmax denominator trick).
     Emitted as PE fillers inside the first attention block.
  3. Attention per (pair, block): 16 t-tile steps. Each step:
     scores[t,s] via 2 row-tiled concurrent 64-contraction matmuls into a
     double-buffered 2-bank PSUM tile; ONE exp ACT [128,1024] evicts both
     heads; ctx accumulation trails by 2 steps (2 matmuls into per-pair ct
     banks, ones column gives the denominator row). Fillers (V proj, next
     block's Q proj, previous block's out proj, normalize) drain one per step.
  4. Normalize: denominator row -> K=1 matmul broadcast -> DVE reciprocal ->
     DVE multiply into persistent ctxT. Out proj [s,D] partial with DVE
     eviction, DMA'd out.
"""
import os
import sys

sys.path.insert(0, "/opt/trn_rl_repo")
os.environ.setdefault("JAX_PLATFORMS", "axon,cpu")

from contextlib import ExitStack

import numpy as np

import concourse.bacc as bacc
import concourse.tile as tile
from concourse import mybir
from concourse.bass_utils import run_bass_kernel_spmd

FP32 = mybir.dt.float32
FP16 = mybir.dt.float16
BF16 = mybir.dt.bfloat16
I16 = mybir.dt.int16

B, S, D = 2, 2048, 1024
H, DK, DV = 16, 64, 64
N_CORES = 8
HPC = H // (N_CORES // B)  # heads per core = 4
P = 128
SBLK = 512                # s-block (free dim of scores matmuls)
NBLK = S // SBLK          # 4
NTT = S // P              # 16 t-tiles
NDC = D // P              # 8 contraction chunks
NV = HPC * (DV + 1)       # 260
SCALE = 1.0 / (DK * 2.0)  # folded into the softmax denominator on device
LAG = 4                   # ctx trails exp by LAG t-tile steps

# Schraudolph fast-exp, bf16 flavor: i16 = round(x * EXP_A16 + EXP_B16) is
# the bit pattern of bf16 ~= exp(x) * (1 + eps), |eps| < ~3% sawtooth. The
# global factor cancels in softmax; measured end-to-end rel err ~2.1e-3.
EXP_A16 = float((1 << 23) / np.log(2.0) / 65536.0)
EXP_B16 = float((127 * (1 << 23) - 486408) / 65536.0)
# steps whose exp runs on the DVE instead of ACT (k % 3 == 1 -> ~1/3)
def _exp_on_dve(k):
    return k % 3 == 1


def _build_nc():
    nc = bacc.Bacc("TRN2", target_bir_lowering=False, debug=False,
                   num_devices=N_CORES)
    d = {}
    for name, shape in [
        ("qt", [D, S]), ("kt", [D, S]), ("vt", [D, S]),
        ("wq", [D, 2 * P]), ("wk", [D, 2 * P]), ("wv", [D, 2 * P]),
        ("wo", [HPC * DV, D]),
    ]:
        d[name] = nc.dram_tensor(name, shape, FP16, kind="ExternalInput").ap()
    d["bqk"] = nc.dram_tensor("bqk", [P, 4], FP32, kind="ExternalInput").ap()
    out_d = nc.dram_tensor("out", [S, D], FP16, kind="ExternalOutput").ap()
    xt_view = {
        n: d[n].rearrange("(dc p) s -> p dc s", p=P)
        for n in ("qt", "kt", "vt")
    }

    with tile.TileContext(nc) as tc, ExitStack() as ctx:
        const = ctx.enter_context(tc.tile_pool(name="const", bufs=1))
        wpool = ctx.enter_context(tc.tile_pool(name="wpool", bufs=1))
        xtp = ctx.enter_context(tc.tile_pool(name="xtp", bufs=4))
        projp = ctx.enter_context(tc.tile_pool(name="projp", bufs=1))
        expp = ctx.enter_context(tc.tile_pool(name="expp", bufs=1))
        ctxp = ctx.enter_context(tc.tile_pool(name="ctxp", bufs=1))
        outp = ctx.enter_context(tc.tile_pool(name="outp", bufs=2))
        smallp = ctx.enter_context(tc.tile_pool(name="smallp", bufs=2))
        psum = ctx.enter_context(tc.tile_pool(name="psum", bufs=1, space="PSUM"))

        # ---- constants / weights (wk first: K projection starts the kernel) ----
        def load_w(sb, name, pat="(dc p) m -> p dc m"):
            nc.sync.dma_start(sb[:], d[name].rearrange(pat, p=P))

        wk_sb = wpool.tile([P, NDC, 2 * P], FP16)
        load_w(wk_sb, "wk")
        bqk = const.tile([P, 4], FP32)
        nc.sync.dma_start(bqk[:], d["bqk"])
        wq_sb = wpool.tile([P, NDC, 2 * P], FP16)
        wv_sb = wpool.tile([P, NDC, 2 * P], FP16)
        wo_sb = wpool.tile([P, 2, D], FP16)

        # ---- persistent activation tiles ----
        qwt = [projp.tile([P, S], FP16, tag=f"qwt{p_}", name=f"qwt{p_}") for p_ in range(2)]
        kwt = [projp.tile([P, S], FP16, tag=f"kwt{p_}", name=f"kwt{p_}") for p_ in range(2)]
        vw = projp.tile([P, NTT, NV], BF16, tag="vw")
        # softmax-denominator ones column (once, strided over the 65-wide head slots)
        for hh in range(HPC):
            nc.vector.memset(vw[:, :, hh * (DV + 1) + DV], 1.0)
        ctx_t = [ctxp.tile([P, S], FP16, tag=f"ctx{p_}", name=f"ctx{p_}") for p_ in range(2)]

        def load_chunk(name, col0, width=SBLK, bufs=1):
            # ONE dma_start per chunk: every DMA costs ~0.7us of sync-engine
            # trigger/semaphore work, which bounds the prologue — keep count low
            xt = xtp.tile([P, NDC, width], FP16, tag=f"xt{width}", name="xt",
                          bufs=bufs)
            nc.sync.dma_start(xt[:], xt_view[name][:, :, col0:col0 + width])
            return xt

        def proj_qk_pair(xt, off, w_sb, dst, bias_col, ci, pair, tag):
            """Project one head-pair of a 512-slice into dst[pair][:, ci*SBLK:...]."""
            pq = psum.tile([P, SBLK], FP32, tag=tag, name="pq")
            for dc in range(NDC):
                nc.tensor.matmul(pq[:], lhsT=w_sb[:, dc, pair * P:(pair + 1) * P],
                                 rhs=xt[:, dc, off:off + SBLK],
                                 start=(dc == 0), stop=(dc == NDC - 1))
            nc.scalar.activation(dst[pair][:, ci * SBLK:(ci + 1) * SBLK], pq[:],
                                 mybir.ActivationFunctionType.Identity,
                                 bias=bqk[:, bias_col + pair:bias_col + pair + 1])

        def proj_qk_piece(xt, off, w_sb, dst, bias_col, ci, pair, dc_range, pq_holder):
            if dc_range[0] == 0:
                pq_holder[pair] = psum.tile([P, SBLK], FP32, tag="pj", name="pq")
            pq = pq_holder[pair]
            for dc in dc_range:
                nc.tensor.matmul(pq[:], lhsT=w_sb[:, dc, pair * P:(pair + 1) * P],
                                 rhs=xt[:, dc, off:off + SBLK],
                                 start=(dc == 0), stop=(dc == NDC - 1))
            if dc_range[-1] == NDC - 1:
                nc.scalar.activation(dst[pair][:, ci * SBLK:(ci + 1) * SBLK], pq[:],
                                     mybir.ActivationFunctionType.Identity,
                                     bias=bqk[:, bias_col + pair:bias_col + pair + 1])

        # V proj, natural orientation: one t-tile per call (8 matmuls, free=256).
        # vp holds 2 t-tiles per PSUM bank; evicted per t-tile by the DVE.
        vp_holder = [None]

        def proj_v_tt(vt_chunk, tt):
            par = tt % 2
            if par == 0:
                vp_holder[0] = psum.tile([P, 2, 2 * P], FP32, tag="po", name="vp")
            vp = vp_holder[0]
            off = (tt * P) % (2 * SBLK)
            for dc in range(NDC):
                nc.tensor.matmul(vp[:, par, :],
                                 lhsT=vt_chunk[:, dc, off:off + P],
                                 rhs=wv_sb[:, dc, :],
                                 start=(dc == 0), stop=(dc == NDC - 1))
            # evict into the 65-wide head slots (dv 0:64 of each slot)
            nc.vector.tensor_copy(
                vw[:, tt, :].rearrange("p (h v) -> p h v", v=DV + 1)[:, :, 0:DV],
                vp[:, par, :].rearrange("p (h v) -> p h v", h=HPC))

        def attn_normalize_hp(pair, b, ct, hp):
            # ctx = ct[0:64] * (SCALE / ct[64]) row-broadcast, one head.
            # SCALE folds into the denominator (den * 1/SCALE) so it never
            # degrades the fp16 weight/activation paths.
            den = smallp.tile([1, SBLK], FP32, tag="den")
            nc.vector.tensor_scalar_mul(den[:], ct[hp][DV:DV + 1, :], 1.0 / SCALE)
            dbc = smallp.tile([DV, SBLK], FP32, tag="dbc")
            nc.gpsimd.partition_broadcast(dbc[:], den[:], channels=DV)
            rcp = smallp.tile([DV, SBLK], FP32, tag="rcp")
            nc.vector.reciprocal_approx_fast(rcp[:], dbc[:])
            nc.vector.tensor_mul(
                ctx_t[pair][hp * DV:(hp + 1) * DV, b * SBLK:(b + 1) * SBLK],
                ct[hp][0:DV, :], rcp[:])

        def norm_fillers(pair, b, ct):
            return [lambda h=hp: attn_normalize_hp(pair, b, ct, h)
                    for hp in range(2)]

        def out_proj_st(b, st, tag="po"):
            # one 128-row slice of the output: both 512-wide halves of D,
            # one batched row DMA out
            off = b * SBLK + st * P
            ob = outp.tile([P, D], FP16, tag="ob")
            for nh in range(2):
                po = psum.tile([P, SBLK], FP32, tag=tag, name="po")
                for jc in range(2):
                    nc.tensor.matmul(po[:],
                                     lhsT=ctx_t[jc][:, off:off + P],
                                     rhs=wo_sb[:, jc, nh * SBLK:(nh + 1) * SBLK],
                                     start=(jc == 0), stop=(jc == 1))
                nc.vector.tensor_copy(ob[:, nh * SBLK:(nh + 1) * SBLK], po[:])
            nc.sync.dma_start(out_d[off:off + P, :], ob[:])

        # ---- prologue: K full, Q block 0 ----
        for ci in range(2):
            kt = load_chunk("kt", ci * 2 * SBLK, width=2 * SBLK, bufs=3)  # 2MB halves
            if ci == 0:
                load_w(wq_sb, "wq")
                load_w(wv_sb, "wv")
            if ci == 1:
                load_w(wo_sb, "wo", "(jc p) n -> p jc n")
            for sub in range(2):
                cc = 2 * ci + sub
                proj_qk_pair(kt, sub * SBLK, wk_sb, kwt, 2, cc, 0, "pj")
                proj_qk_pair(kt, sub * SBLK, wk_sb, kwt, 2, cc, 1, "po")
        qt0 = load_chunk("qt", 0)
        proj_qk_pair(qt0, 0, wq_sb, qwt, 0, 0, 0, "pj")
        proj_qk_pair(qt0, 0, wq_sb, qwt, 0, 0, 1, "po")
        # vt halves stream in during the first attention block (same rotating
        # tag as the kt halves)
        vts = [load_chunk("vt", 0, width=2 * SBLK, bufs=3)]

        def interleave(a, bl):
            out = []
            for i in range(max(len(a), len(bl))):
                if i < len(a):
                    out.append(a[i])
                if i < len(bl):
                    out.append(bl[i])
            return out

        # ---- attention: (pair, block) segments of 16 t-tile steps ----
        def attn_segment(pair, b, ct, fillers):
            """scores(tt) -> exp(tt) -> ctx(tt-LAG), one filler per step."""
            exs = {}
            for k in range(NTT + LAG):
                if k < NTT:
                    tt = k
                    sc = psum.tile([P, 2, SBLK], FP32, tag="sc", name="sc", bufs=2)
                    for hp in range(2):
                        lo, hi = hp * DK, (hp + 1) * DK
                        nc.tensor.matmul(
                            sc[:, hp, :],
                            lhsT=kwt[pair][lo:hi, tt * P:(tt + 1) * P],
                            rhs=qwt[pair][lo:hi, b * SBLK:(b + 1) * SBLK],
                            start=True, stop=True)
                    ex = expp.tile([P, 2, SBLK], BF16, tag="exp", name="ex", bufs=LAG + 3)
                    if _exp_on_dve(k):
                        # fused fast-exp: int16(x*A + B) is bf16 ~= exp(x)
                        nc.vector.tensor_scalar(
                            ex[:].bitcast(I16), sc[:], EXP_A16, EXP_B16,
                            mybir.AluOpType.mult, mybir.AluOpType.add)
                    else:
                        nc.scalar.activation(ex[:], sc[:],
                                             mybir.ActivationFunctionType.Exp)
                    exs[k] = ex
                if fillers:
                    fillers.pop(0)()
                kc = k - LAG
                if kc >= 0:
                    ex = exs.pop(kc)
                    for hp in range(2):
                        hh = 2 * pair + hp
                        nc.tensor.matmul(
                            ct[hp][:], lhsT=vw[:, kc, hh * (DV + 1):(hh + 1) * (DV + 1)],
                            rhs=ex[:, hp, :],
                            start=(kc == 0), stop=(kc == NTT - 1))

        def attn_alloc():
            return [psum.tile([DV + 1, SBLK], FP32, tag=f"ct{hp}", name=f"ct{hp}")
                    for hp in range(2)]

        def v_filler(tt):
            def go():
                if tt == 0:
                    vts.append(load_chunk("vt", 2 * SBLK, width=2 * SBLK, bufs=3))
                proj_v_tt(vts[tt // 8], tt)
            return go

        qt_rest = [None]  # blocks 1-3 of qt, one 3MB load
        prev_norm = []    # pair-1 normalize deferred into the next block
        for b in range(NBLK):
            have_next = b + 1 < NBLK
            # fillers for pair 0 segment: deferred normalize first (ct banks
            # are reused by this segment's ctx at step LAG)
            fill0 = list(prev_norm)
            prev_norm = []
            if b == 0:
                # V projection: all 16 t-tiles as fillers here — filler tt
                # runs at step tt, ctx(tt) consumes vw[tt] at step tt+LAG
                fill0 += [v_filler(tt) for tt in range(NTT)]
            else:
                fill0 += [lambda s=st, bb=b: out_proj_st(bb - 1, s)
                          for st in range(2)]
            ct = attn_alloc()
            attn_segment(0, b, ct, fill0)

            # fillers for pair 1 segment: normalize(0,b) first, then q-proj/
            # out-proj pieces
            fill1 = norm_fillers(0, b, ct)
            pp = []
            if have_next:
                if qt_rest[0] is None:
                    # qt columns SBLK..4*SBLK (blocks 1-3), one 3MB load
                    qt_rest[0] = load_chunk("qt", SBLK, width=3 * SBLK)
                holder = [None, None]
                for pair_ in range(2):
                    for dcs in ([0, 1], [2, 3], [4, 5], [6, 7]):
                        pp.append(lambda p=pair_, r=tuple(dcs), h=holder, bb=b:
                                  proj_qk_piece(qt_rest[0], bb * SBLK,
                                                wq_sb, qwt, 0, bb + 1, p, r, h))
            op = []
            if b > 0:
                op += [lambda s=st, bb=b: out_proj_st(bb - 1, s)
                       for st in range(2, 4)]
            ct1 = attn_alloc()
            attn_segment(1, b, ct1, fill1 + interleave(pp, op))
            prev_norm = norm_fillers(1, b, ct1)
        for f in prev_norm:
            f()
        for st in range(4):
            out_proj_st(NBLK - 1, st, tag="po" if st % 2 == 0 else "pj")

    nc.compile()
    return nc


_NC_CACHE = None


def _get_nc():
    global _NC_CACHE
    if _NC_CACHE is None:
        _NC_CACHE = _build_nc()
    return _NC_CACHE


def kernel(Q, K, V, Wq, bq, Wk, bk, Wv, bv, Wo, bo, _trace=False, _trace_kwargs=None):
    nc = _get_nc()
    f16 = np.float16
    qt_h = [np.ascontiguousarray(np.asarray(Q[b]).T.astype(f16)) for b in range(B)]
    kt_h = [np.ascontiguousarray(np.asarray(K[b]).T.astype(f16)) for b in range(B)]
    vt_h = [np.ascontiguousarray(np.asarray(V[b]).T.astype(f16)) for b in range(B)]

    in_maps = []
    for c in range(N_CORES):
        b, g = c % B, c // B
        hs = list(range(g * HPC, (g + 1) * HPC))
        wq_p = np.concatenate([Wq[h] for h in hs], axis=1)
        wk_p = np.concatenate([Wk[h] for h in hs], axis=1)
        wv_p = np.concatenate([Wv[h] for h in hs], axis=1)
        bqk_p = np.stack([
            np.concatenate([bq[hs[0]], bq[hs[1]]]),
            np.concatenate([bq[hs[2]], bq[hs[3]]]),
            np.concatenate([bk[hs[0]], bk[hs[1]]]),
            np.concatenate([bk[hs[2]], bk[hs[3]]]),
        ], axis=1)
        in_maps.append({
            "qt": qt_h[b], "kt": kt_h[b], "vt": vt_h[b],
            "wq": np.ascontiguousarray(wq_p.astype(f16)),
            "wk": np.ascontiguousarray(wk_p.astype(f16)),
            "wv": np.ascontiguousarray(wv_p.astype(f16)),
            "bqk": np.ascontiguousarray(bqk_p.astype(np.float32)),
            "wo": np.ascontiguousarray(
                Wo[g * HPC * DV:(g + 1) * HPC * DV].astype(f16)),
        })

    kw = {}
    if _trace:
        kw = dict(trace=True, **(_trace_kwargs or {}))
    res = run_bass_kernel_spmd(nc, in_maps, core_ids=list(range(N_CORES)), **kw)

    out = np.zeros((B, S, D), dtype=np.float32)
    for c in range(N_CORES):
        out[c % B] += res.results[c]["out"].astype(np.float32)
    # host-side constant terms: output bias + V-bias (softmax weights sum to 1,
    # so the V bias contributes SCALE * bv @ Wo, constant over (b, s))
    out += bo[None, None, :] + (SCALE * bv.reshape(-1)) @ Wo
    if _trace:
        return out, res
    return out
